# revision 2
# baseline (speedup 1.0000x reference)
"""Trainium2 Bass kernel for the AttentionLayer problem.

Computation (per batch b):
    keys' = keys + sinenc(text_pos, w=1.385);  query' = query + sinenc(frame_pos, w=1.0)
    q = query' @ Wq + bq ; k = keys' @ Wk + bk ; v = values @ Wv + bv
    scores = q @ k^T ; masked softmax over keys -> attn  (output 1)
    out = (attn @ v) * sqrt(1/512) @ Wo + bo             (output 2)

Device strategy: data-parallel over B=64 across 8 cores (8 batches/core).
All matmuls run in float32r (full PE throughput, ~1.6e-4 rel precision).
Everything is computed in a transposed layout ([feature, time]) so that no
on-device transposes are needed anywhere:
    qT = Wq^T @ query'^T          kT = Wk^T @ keys'^T     v = values'^T^T... (v natural)
    scoresT[k,q] = kT^T @ qT      exp via ACT(Exp, bias=mask_bias[k])
    denom[q] = ones^T @ expT      attnT = expT * (1/denom)
    xT[h,q] = v^T @ attnT         outT[c,q] = Wo'^T @ xT (+ bo')
Host pre-transposes inputs and post-transposes outputs; the sqrt scale is
folded into Wo, the value bias bv is folded into the output bias via
bo' = s*bv@Wo + bo (valid because attn rows sum to 1).
"""

import math
import os

import numpy as np

import concourse.bass as bass
import concourse.tile as tile
from concourse import bacc, mybir
from concourse.bass_utils import run_bass_kernel_spmd

dt = mybir.dt
F32 = dt.float32
F32R = dt.float32r
AF = mybir.ActivationFunctionType

B, TQ, TK = 64, 1024, 512
CH = 512          # conv_channels == embed_dim == att_hid
N_CORES = 8
BPC = B // N_CORES  # batches per core
KEY_POS_RATE = 1.385
QUERY_POS_RATE = 1.0
OUT_SCALE = math.sqrt(1.0 / TK)
MASK_NEG = -1.0e30

_LAST_EXEC_NS = None


def _sin_pos_enc(pos, w, d):
    """Reference-exact sinusoidal table for one position vector. [T, d] f32."""
    pos = pos.astype(np.float32)
    i = np.arange(d)
    inv_freq = np.power(np.float32(10000.0), -(2.0 * (i // 2)).astype(np.float32) / d)
    ang = (pos * np.float32(w))[:, None] * inv_freq[None, :]
    pe = np.where(i[None, :] % 2 == 0, np.sin(ang), np.cos(ang)).astype(np.float32)
    pe[pos == 0] = 0.0
    return pe


def _build_program(n_batch, pe_tabs_q, pe_tabs_k):
    """One-core program; pe_tabs_* is 1 (shared tables) or n_batch."""
    nc = bacc.Bacc("TRN2", target_bir_lowering=False, debug=False, num_devices=1)

    qT_d = nc.dram_tensor("qT", [n_batch, CH, TQ], F32R, kind="ExternalInput")
    kT_d = nc.dram_tensor("kT", [n_batch, CH, TK], F32R, kind="ExternalInput")
    vT_d = nc.dram_tensor("vT", [n_batch, CH, TK], F32R, kind="ExternalInput")
    peq_d = nc.dram_tensor("peq", [pe_tabs_q, CH, TQ], F32R, kind="ExternalInput")
    pek_d = nc.dram_tensor("pek", [pe_tabs_k, CH, TK], F32R, kind="ExternalInput")
    wq_d = nc.dram_tensor("wq", [CH, CH], F32R, kind="ExternalInput")
    wk_d = nc.dram_tensor("wk", [CH, CH], F32R, kind="ExternalInput")
    wv_d = nc.dram_tensor("wv", [CH, CH], F32R, kind="ExternalInput")
    wo_d = nc.dram_tensor("wo", [CH, CH], F32R, kind="ExternalInput")
    bq_d = nc.dram_tensor("bq", [CH], F32, kind="ExternalInput")
    bk_d = nc.dram_tensor("bk", [CH], F32, kind="ExternalInput")
    bo_d = nc.dram_tensor("bo", [CH], F32, kind="ExternalInput")
    mb_d = nc.dram_tensor("mb", [n_batch, TK], F32, kind="ExternalInput")
    ones_d = nc.dram_tensor("ones", [128, 128], F32R, kind="ExternalInput")

    attn_d = nc.dram_tensor("attnT", [n_batch, TK, TQ], F32, kind="ExternalOutput")
    out_d = nc.dram_tensor("outT", [n_batch, CH, TQ], F32, kind="ExternalOutput")

    NC2, NQ2 = TK // 512, TQ // 512   # 512-wide chunks: 1, 2
    NCT = CH // 128                   # 4 feature tiles
    NKT = TK // 128                   # 4 key tiles
    s512 = lambda c: slice(c * 512, (c + 1) * 512)
    s128 = lambda t: slice(t * 128, (t + 1) * 128)

    with tile.TileContext(nc) as tc:
        with (
            tc.tile_pool(name="wpool", bufs=1) as wpool,
            tc.tile_pool(name="qin", bufs=5) as p_qin,
            tc.tile_pool(name="kin", bufs=5) as p_kin,
            tc.tile_pool(name="vin", bufs=5) as p_vin,
            tc.tile_pool(name="qt", bufs=5) as p_qt,
            tc.tile_pool(name="kt", bufs=5) as p_kt,
            tc.tile_pool(name="vt", bufs=5) as p_vt,
            tc.tile_pool(name="exp", bufs=5) as p_exp,
            tc.tile_pool(name="rec", bufs=2) as p_rec,
            tc.tile_pool(name="attn", bufs=5) as p_attn,
            tc.tile_pool(name="xt", bufs=5) as p_xt,
            tc.tile_pool(name="outt", bufs=3) as p_out,
            tc.tile_pool(name="mb", bufs=2) as p_mb,
            tc.tile_pool(name="ps", bufs=8, space="PSUM") as p_ps,
        ):
            # ---- resident weights/constants ----
            def load_w(name, dram):
                ts = []
                for ct in range(NCT):
                    t = wpool.tile([128, CH], F32R, name=f"{name}{ct}")
                    nc.sync.dma_start(t[:], dram.ap()[s128(ct), :])
                    ts.append(t)
                return ts

            wq = load_w("wq", wq_d)
            wk = load_w("wk", wk_d)
            wv = load_w("wv", wv_d)
            wo = load_w("wo", wo_d)
            ones_sb = wpool.tile([128, 128], F32R, name="ones")
            nc.sync.dma_start(ones_sb[:], ones_d.ap())

            def load_bias(name, dram):
                t = wpool.tile([128, NCT], F32, name=name)
                nc.sync.dma_start(
                    t[:], dram.ap().rearrange("(j p) -> p j", p=128)
                )
                return t

            bq_sb = load_bias("bqc", bq_d)
            bk_sb = load_bias("bkc", bk_d)
            bo_sb = load_bias("boc", bo_d)

            ps_t = lambda nm: p_ps.tile([128, 512], F32, name=nm, tag="ps")

            for b in range(n_batch):
                tq = b if pe_tabs_q > 1 else 0
                tk_ = b if pe_tabs_k > 1 else 0

                # ---- inputs (+ positional bias via DMA-accumulate) ----
                qin = []
                for ct in range(NCT):
                    t = p_qin.tile([128, TQ], F32R, name=f"qin{b}_{ct}", tag="qin")
                    nc.sync.dma_start(t[:], qT_d.ap()[b, s128(ct), :])
                    nc.gpsimd.dma_start(
                        t[:], peq_d.ap()[tq, s128(ct), :],
                        accum_op=mybir.AluOpType.add,
                    )
                    qin.append(t)
                kin = []
                for ct in range(NCT):
                    t = p_kin.tile([128, TK], F32R, name=f"kin{b}_{ct}", tag="kin")
                    nc.sync.dma_start(t[:], kT_d.ap()[b, s128(ct), :])
                    nc.gpsimd.dma_start(
                        t[:], pek_d.ap()[tk_, s128(ct), :],
                        accum_op=mybir.AluOpType.add,
                    )
                    kin.append(t)
                vin = []
                for ct in range(NCT):
                    t = p_vin.tile([128, TK], F32R, name=f"vin{b}_{ct}", tag="vin")
                    nc.sync.dma_start(t[:], vT_d.ap()[b, s128(ct), :])
                    vin.append(t)
                mb_t = p_mb.tile([128, NKT], F32, name=f"mb{b}", tag="mb")
                nc.sync.dma_start(
                    mb_t[:], mb_d.ap()[b].rearrange("(j p) -> p j", p=128)
                )

                # ---- projections ----
                qt = []
                for ht in range(NCT):
                    ps = [ps_t(f"psq{b}_{ht}_{c}") for c in range(NQ2)]
                    for ct in range(NCT):
                        for c in range(NQ2):
                            nc.tensor.matmul(
                                ps[c][:], wq[ct][:, s128(ht)], qin[ct][:, s512(c)],
                                start=(ct == 0), stop=(ct == NCT - 1),
                            )
                    t = p_qt.tile([128, TQ], F32R, name=f"qt{b}_{ht}", tag="qt")
                    for c in range(NQ2):
                        nc.scalar.activation(
                            t[:, s512(c)], ps[c][:], AF.Identity,
                            bias=bq_sb[:, ht:ht + 1],
                        )
                    qt.append(t)
                kt = []
                for ht in range(NCT):
                    ps = [ps_t(f"psk{b}_{ht}_{c}") for c in range(NC2)]
                    for ct in range(NCT):
                        for c in range(NC2):
                            nc.tensor.matmul(
                                ps[c][:], wk[ct][:, s128(ht)], kin[ct][:, s512(c)],
                                start=(ct == 0), stop=(ct == NCT - 1),
                            )
                    t = p_kt.tile([128, TK], F32R, name=f"kt{b}_{ht}", tag="kt")
                    for c in range(NC2):
                        nc.scalar.activation(
                            t[:, s512(c)], ps[c][:], AF.Identity,
                            bias=bk_sb[:, ht:ht + 1],
                        )
                    kt.append(t)
                vt = []
                for ktile in range(NKT):
                    ps = ps_t(f"psv{b}_{ktile}")
                    for ct in range(NCT):
                        nc.tensor.matmul(
                            ps[:], vin[ct][:, s128(ktile)], wv[ct][:],
                            start=(ct == 0), stop=(ct == NCT - 1),
                        )
                    t = p_vt.tile([128, CH], F32R, name=f"vt{b}_{ktile}", tag="vt")
                    nc.scalar.copy(t[:], ps[:])
                    vt.append(t)

                # ---- scores + exp (mask folded into bias) ----
                expt = []
                for ktile in range(NKT):
                    ps = [ps_t(f"pss{b}_{ktile}_{c}") for c in range(NQ2)]
                    for ht in range(NCT):
                        for c in range(NQ2):
                            nc.tensor.matmul(
                                ps[c][:], kt[ht][:, s128(ktile)], qt[ht][:, s512(c)],
                                start=(ht == 0), stop=(ht == NCT - 1),
                            )
                    t = p_exp.tile([128, TQ], F32R, name=f"exp{b}_{ktile}", tag="exp")
                    for c in range(NQ2):
                        nc.scalar.activation(
                            t[:, s512(c)], ps[c][:], AF.Exp,
                            bias=mb_t[:, ktile:ktile + 1],
                        )
                    expt.append(t)

                # ---- softmax denominators (all-ones matmul), reciprocal ----
                rec = p_rec.tile([128, TQ], F32R, name=f"rec{b}", tag="rec")
                for c in range(NQ2):
                    ps = ps_t(f"pssum{b}_{c}")
                    for ktile in range(NKT):
                        nc.tensor.matmul(
                            ps[:], ones_sb[:], expt[ktile][:, s512(c)],
                            start=(ktile == 0), stop=(ktile == NKT - 1),
                        )
                    with nc.allow_low_precision(
                        reason="f32r denominators feed f32r matmuls"
                    ):
                        nc.vector.reciprocal(rec[:, s512(c)], ps[:])

                # ---- attn (normalized) -> DRAM + x ----
                attn = []
                for ktile in range(NKT):
                    t = p_attn.tile([128, TQ], F32R, name=f"at{b}_{ktile}", tag="attn")
                    nc.vector.tensor_mul(t[:], expt[ktile][:], rec[:])
                    nc.sync.dma_start(
                        attn_d.ap()[b, s128(ktile), :], t[:].bitcast(F32)
                    )
                    attn.append(t)

                xt = []
                for ht in range(NCT):
                    ps = [ps_t(f"psx{b}_{ht}_{c}") for c in range(NQ2)]
                    for ktile in range(NKT):
                        for c in range(NQ2):
                            nc.tensor.matmul(
                                ps[c][:], vt[ktile][:, s128(ht)],
                                attn[ktile][:, s512(c)],
                                start=(ktile == 0), stop=(ktile == NKT - 1),
                            )
                    t = p_xt.tile([128, TQ], F32R, name=f"xt{b}_{ht}", tag="xt")
                    for c in range(NQ2):
                        nc.vector.tensor_copy(t[:, s512(c)], ps[c][:])
                    xt.append(t)

                for ct in range(NCT):
                    ps = [ps_t(f"pso{b}_{ct}_{c}") for c in range(NQ2)]
                    for ht in range(NCT):
                        for c in range(NQ2):
                            nc.tensor.matmul(
                                ps[c][:], wo[ht][:, s128(ct)], xt[ht][:, s512(c)],
                                start=(ht == 0), stop=(ht == NCT - 1),
                            )
                    t = p_out.tile([128, TQ], F32, name=f"ot{b}_{ct}", tag="outt")
                    for c in range(NQ2):
                        nc.scalar.activation(
                            t[:, s512(c)], ps[c][:], AF.Identity,
                            bias=bo_sb[:, ct:ct + 1],
                        )
                    nc.sync.dma_start(out_d.ap()[b, s128(ct), :], t[:])

    nc.compile()
    return nc


def _host_prep(inputs):
    query = np.asarray(inputs["query"], dtype=np.float32)
    keys = np.asarray(inputs["keys"], dtype=np.float32)
    values = np.asarray(inputs["values"], dtype=np.float32)
    tpos = np.asarray(inputs["text_positions"])
    fpos = np.asarray(inputs["frame_positions"])
    mask = np.asarray(inputs["mask"])
    Wq = np.asarray(inputs["Wq"], dtype=np.float32)
    Wk = np.asarray(inputs["Wk"], dtype=np.float32)
    Wv = np.asarray(inputs["Wv"], dtype=np.float32)
    Wo = np.asarray(inputs["Wo"], dtype=np.float32)
    bq = np.asarray(inputs["bq"], dtype=np.float32)
    bk = np.asarray(inputs["bk"], dtype=np.float32)
    bv = np.asarray(inputs["bv"], dtype=np.float32)
    bo = np.asarray(inputs["bo"], dtype=np.float32)

    qT = np.ascontiguousarray(query.transpose(0, 2, 1))
    kT = np.ascontiguousarray(keys.transpose(0, 2, 1))
    vT = np.ascontiguousarray(values.transpose(0, 2, 1))

    # positional-encoding tables (shared across batch when positions agree)
    fshared = bool(np.all(fpos == fpos[0:1]))
    tshared = bool(np.all(tpos == tpos[0:1]))
    fp = fpos[0:1] if fshared else fpos
    tp = tpos[0:1] if tshared else tpos
    peq = np.stack([np.ascontiguousarray(_sin_pos_enc(p, QUERY_POS_RATE, CH).T)
                    for p in fp])
    pek = np.stack([np.ascontiguousarray(_sin_pos_enc(p, KEY_POS_RATE, CH).T)
                    for p in tp])

    mb = np.where(mask, np.float32(MASK_NEG), np.float32(0.0)).astype(np.float32)
    wo_s = (Wo * np.float32(OUT_SCALE)).astype(np.float32)
    bo_s = (np.float32(OUT_SCALE) * (bv @ Wo) + bo).astype(np.float32)
    ones = np.ones((128, 128), dtype=np.float32)

    shared = {
        "wq": Wq, "wk": Wk, "wv": Wv, "wo": wo_s,
        "bq": bq, "bk": bk, "bo": bo_s, "ones": ones,
    }
    in_maps = []
    for c in range(N_CORES):
        sl = slice(c * BPC, (c + 1) * BPC)
        m = dict(shared)
        m["qT"] = qT[sl]
        m["kT"] = kT[sl]
        m["vT"] = vT[sl]
        m["peq"] = peq if fshared else peq[sl]
        m["pek"] = pek if tshared else pek[sl]
        m["mb"] = mb[sl]
        in_maps.append(m)
    return in_maps, fshared, tshared


def kernel(**inputs):
    global _LAST_EXEC_NS
    in_maps, fshared, tshared = _host_prep(inputs)
    nc = _build_program(
        BPC,
        1 if fshared else BPC,
        1 if tshared else BPC,
    )
    trace = bool(int(os.environ.get("KERNEL_PROFILE", "0")))
    res = run_bass_kernel_spmd(nc, in_maps, list(range(N_CORES)), trace=trace)
    _LAST_EXEC_NS = res.exec_time_ns

    attn = np.empty((B, TQ, TK), dtype=np.float32)
    out = np.empty((B, TQ, CH), dtype=np.float32)
    for c in range(N_CORES):
        r = res.results[c]
        sl = slice(c * BPC, (c + 1) * BPC)
        attn[sl] = r["attnT"].transpose(0, 2, 1)
        out[sl] = r["outT"].transpose(0, 2, 1)
    return out, attn


# revision 3
# speedup vs baseline: 1.1752x; 1.1752x over previous
"""Trainium2 Bass kernel for the AttentionLayer problem.

Computation (per batch b):
    keys' = keys + sinenc(text_pos, w=1.385);  query' = query + sinenc(frame_pos, w=1.0)
    q = query' @ Wq + bq ; k = keys' @ Wk + bk ; v = values @ Wv + bv
    scores = q @ k^T ; masked softmax over keys -> attn  (output 1)
    out = (attn @ v) * sqrt(1/512) @ Wo + bo             (output 2)

Device strategy: data-parallel over B=64 across 8 cores (8 batches/core).
All matmuls run in float32r (full PE throughput, ~1.6e-4 rel precision).
Everything is computed in a transposed layout ([feature, time]) so that no
on-device transposes are needed anywhere:
    qT = Wq^T @ query'^T          kT = Wk^T @ keys'^T     v = values'^T^T... (v natural)
    scoresT[k,q] = kT^T @ qT      exp via ACT(Exp, bias=mask_bias[k])
    denom[q] = ones^T @ expT      attnT = expT * (1/denom)
    xT[h,q] = v^T @ attnT         outT[c,q] = Wo'^T @ xT (+ bo')
Host pre-transposes inputs and post-transposes outputs; the sqrt scale is
folded into Wo, the value bias bv is folded into the output bias via
bo' = s*bv@Wo + bo (valid because attn rows sum to 1).
"""

import math
import os

import numpy as np

import concourse.bass as bass
import concourse.tile as tile
from concourse import bacc, mybir
from concourse.bass_utils import run_bass_kernel_spmd

dt = mybir.dt
F32 = dt.float32
F32R = dt.float32r
AF = mybir.ActivationFunctionType

B, TQ, TK = 64, 1024, 512
CH = 512          # conv_channels == embed_dim == att_hid
N_CORES = 8
BPC = B // N_CORES  # batches per core
KEY_POS_RATE = 1.385
QUERY_POS_RATE = 1.0
OUT_SCALE = math.sqrt(1.0 / TK)
MASK_NEG = -1.0e30

_LAST_EXEC_NS = None
_LAST_RES = None


def _sin_pos_enc(pos, w, d):
    """Reference-exact sinusoidal table for one position vector. [T, d] f32."""
    pos = pos.astype(np.float32)
    i = np.arange(d)
    inv_freq = np.power(np.float32(10000.0), -(2.0 * (i // 2)).astype(np.float32) / d)
    ang = (pos * np.float32(w))[:, None] * inv_freq[None, :]
    pe = np.where(i[None, :] % 2 == 0, np.sin(ang), np.cos(ang)).astype(np.float32)
    pe[pos == 0] = 0.0
    return pe


def _build_program(n_batch, pe_tabs_q, pe_tabs_k):
    """One-core program; pe_tabs_* is 1 (shared tables) or n_batch."""
    nc = bacc.Bacc("TRN2", target_bir_lowering=False, debug=False, num_devices=1)

    qT_d = nc.dram_tensor("qT", [n_batch, CH, TQ], F32R, kind="ExternalInput")
    kT_d = nc.dram_tensor("kT", [n_batch, CH, TK], F32R, kind="ExternalInput")
    vT_d = nc.dram_tensor("vT", [n_batch, CH, TK], F32R, kind="ExternalInput")
    peq_d = nc.dram_tensor("peq", [pe_tabs_q, CH, TQ], F32R, kind="ExternalInput")
    pek_d = nc.dram_tensor("pek", [pe_tabs_k, CH, TK], F32R, kind="ExternalInput")
    wq_d = nc.dram_tensor("wq", [CH, CH], F32R, kind="ExternalInput")
    wk_d = nc.dram_tensor("wk", [CH, CH], F32R, kind="ExternalInput")
    wv_d = nc.dram_tensor("wv", [CH, CH], F32R, kind="ExternalInput")
    wo_d = nc.dram_tensor("wo", [CH, CH], F32R, kind="ExternalInput")
    bq_d = nc.dram_tensor("bq", [CH], F32, kind="ExternalInput")
    bk_d = nc.dram_tensor("bk", [CH], F32, kind="ExternalInput")
    bo_d = nc.dram_tensor("bo", [CH], F32, kind="ExternalInput")
    mb_d = nc.dram_tensor("mb", [n_batch, TK], F32, kind="ExternalInput")
    ones_d = nc.dram_tensor("ones", [128, 128], F32R, kind="ExternalInput")

    attn_d = nc.dram_tensor("attnT", [n_batch, TK, TQ], F32, kind="ExternalOutput")
    out_d = nc.dram_tensor("outT", [n_batch, CH, TQ], F32, kind="ExternalOutput")

    NC2, NQ2 = TK // 512, TQ // 512   # 512-wide chunks: 1, 2
    NCT = CH // 128                   # 4 feature tiles
    NKT = TK // 128                   # 4 key tiles
    s512 = lambda c: slice(c * 512, (c + 1) * 512)
    s128 = lambda t: slice(t * 128, (t + 1) * 128)

    with tile.TileContext(nc) as tc:
        with (
            tc.tile_pool(name="wpool", bufs=1) as wpool,
            tc.tile_pool(name="qin", bufs=5) as p_qin,
            tc.tile_pool(name="kin", bufs=5) as p_kin,
            tc.tile_pool(name="vin", bufs=5) as p_vin,
            tc.tile_pool(name="qt", bufs=5) as p_qt,
            tc.tile_pool(name="kt", bufs=5) as p_kt,
            tc.tile_pool(name="vt", bufs=5) as p_vt,
            tc.tile_pool(name="exp", bufs=5) as p_exp,
            tc.tile_pool(name="rec", bufs=2) as p_rec,
            tc.tile_pool(name="attn", bufs=5) as p_attn,
            tc.tile_pool(name="xt", bufs=5) as p_xt,
            tc.tile_pool(name="outt", bufs=3) as p_out,
            tc.tile_pool(name="mb", bufs=2) as p_mb,
            tc.tile_pool(name="ps", bufs=8, space="PSUM") as p_ps,
        ):
            # ---- resident weights/constants ----
            def load_w(name, dram):
                ts = []
                for ct in range(NCT):
                    t = wpool.tile([128, CH], F32R, name=f"{name}{ct}")
                    nc.sync.dma_start(t[:], dram.ap()[s128(ct), :])
                    ts.append(t)
                return ts

            wq = load_w("wq", wq_d)
            wk = load_w("wk", wk_d)
            wv = load_w("wv", wv_d)
            wo = load_w("wo", wo_d)
            ones_sb = wpool.tile([128, 128], F32R, name="ones")
            nc.sync.dma_start(ones_sb[:], ones_d.ap())

            def load_bias(name, dram):
                t = wpool.tile([128, NCT], F32, name=name)
                nc.sync.dma_start(
                    t[:], dram.ap().rearrange("(j p) -> p j", p=128)
                )
                return t

            bq_sb = load_bias("bqc", bq_d)
            bk_sb = load_bias("bkc", bk_d)
            bo_sb = load_bias("boc", bo_d)

            ps_t = lambda nm: p_ps.tile([128, 512], F32, name=nm, tag="ps")

            for b in range(n_batch):
                tq = b if pe_tabs_q > 1 else 0
                tk_ = b if pe_tabs_k > 1 else 0

                # ---- inputs (+ positional bias via DMA-accumulate) ----
                qin = []
                for ct in range(NCT):
                    t = p_qin.tile([128, TQ], F32R, name=f"qin{b}_{ct}", tag="qin")
                    nc.sync.dma_start(t[:], qT_d.ap()[b, s128(ct), :])
                    nc.gpsimd.dma_start(
                        t[:], peq_d.ap()[tq, s128(ct), :],
                        accum_op=mybir.AluOpType.add,
                    )
                    qin.append(t)
                kin = []
                for ct in range(NCT):
                    t = p_kin.tile([128, TK], F32R, name=f"kin{b}_{ct}", tag="kin")
                    nc.sync.dma_start(t[:], kT_d.ap()[b, s128(ct), :])
                    nc.gpsimd.dma_start(
                        t[:], pek_d.ap()[tk_, s128(ct), :],
                        accum_op=mybir.AluOpType.add,
                    )
                    kin.append(t)
                vin = []
                for ct in range(NCT):
                    t = p_vin.tile([128, TK], F32R, name=f"vin{b}_{ct}", tag="vin")
                    nc.sync.dma_start(t[:], vT_d.ap()[b, s128(ct), :])
                    vin.append(t)
                mb_t = p_mb.tile([128, NKT], F32, name=f"mb{b}", tag="mb")
                nc.sync.dma_start(
                    mb_t[:], mb_d.ap()[b].rearrange("(j p) -> p j", p=128)
                )

                # ---- projections ----
                qt = []
                for ht in range(NCT):
                    ps = [ps_t(f"psq{b}_{ht}_{c}") for c in range(NQ2)]
                    for ct in range(NCT):
                        for c in range(NQ2):
                            nc.tensor.matmul(
                                ps[c][:], wq[ct][:, s128(ht)], qin[ct][:, s512(c)],
                                start=(ct == 0), stop=(ct == NCT - 1),
                            )
                    t = p_qt.tile([128, TQ], F32R, name=f"qt{b}_{ht}", tag="qt")
                    for c in range(NQ2):
                        nc.scalar.activation(
                            t[:, s512(c)], ps[c][:], AF.Identity,
                            bias=bq_sb[:, ht:ht + 1],
                        )
                    qt.append(t)
                kt = []
                for ht in range(NCT):
                    ps = [ps_t(f"psk{b}_{ht}_{c}") for c in range(NC2)]
                    for ct in range(NCT):
                        for c in range(NC2):
                            nc.tensor.matmul(
                                ps[c][:], wk[ct][:, s128(ht)], kin[ct][:, s512(c)],
                                start=(ct == 0), stop=(ct == NCT - 1),
                            )
                    t = p_kt.tile([128, TK], F32R, name=f"kt{b}_{ht}", tag="kt")
                    for c in range(NC2):
                        nc.scalar.activation(
                            t[:, s512(c)], ps[c][:], AF.Identity,
                            bias=bk_sb[:, ht:ht + 1],
                        )
                    kt.append(t)
                vt = []
                for ktile in range(NKT):
                    ps = ps_t(f"psv{b}_{ktile}")
                    for ct in range(NCT):
                        nc.tensor.matmul(
                            ps[:], vin[ct][:, s128(ktile)], wv[ct][:],
                            start=(ct == 0), stop=(ct == NCT - 1),
                        )
                    t = p_vt.tile([128, CH], F32R, name=f"vt{b}_{ktile}", tag="vt")
                    nc.scalar.copy(t[:], ps[:])
                    vt.append(t)

                # ---- scores + exp (mask folded into bias) ----
                expt = []
                for ktile in range(NKT):
                    ps = [ps_t(f"pss{b}_{ktile}_{c}") for c in range(NQ2)]
                    for ht in range(NCT):
                        for c in range(NQ2):
                            nc.tensor.matmul(
                                ps[c][:], kt[ht][:, s128(ktile)], qt[ht][:, s512(c)],
                                start=(ht == 0), stop=(ht == NCT - 1),
                            )
                    t = p_exp.tile([128, TQ], F32R, name=f"exp{b}_{ktile}", tag="exp")
                    for c in range(NQ2):
                        nc.scalar.activation(
                            t[:, s512(c)], ps[c][:], AF.Exp,
                            bias=mb_t[:, ktile:ktile + 1],
                        )
                    expt.append(t)

                # ---- softmax denominators (all-ones matmul), reciprocal ----
                rec = p_rec.tile([128, TQ], F32R, name=f"rec{b}", tag="rec")
                for c in range(NQ2):
                    ps = ps_t(f"pssum{b}_{c}")
                    for ktile in range(NKT):
                        nc.tensor.matmul(
                            ps[:], ones_sb[:], expt[ktile][:, s512(c)],
                            start=(ktile == 0), stop=(ktile == NKT - 1),
                        )
                    with nc.allow_low_precision(
                        reason="f32r denominators feed f32r matmuls"
                    ):
                        nc.vector.reciprocal(rec[:, s512(c)], ps[:])

                # ---- attn (normalized) -> DRAM + x ----
                attn = []
                for ktile in range(NKT):
                    t = p_attn.tile([128, TQ], F32R, name=f"at{b}_{ktile}", tag="attn")
                    nc.vector.tensor_mul(t[:], expt[ktile][:], rec[:])
                    nc.sync.dma_start(
                        attn_d.ap()[b, s128(ktile), :], t[:].bitcast(F32)
                    )
                    attn.append(t)

                xt = []
                for ht in range(NCT):
                    ps = [ps_t(f"psx{b}_{ht}_{c}") for c in range(NQ2)]
                    for ktile in range(NKT):
                        for c in range(NQ2):
                            nc.tensor.matmul(
                                ps[c][:], vt[ktile][:, s128(ht)],
                                attn[ktile][:, s512(c)],
                                start=(ktile == 0), stop=(ktile == NKT - 1),
                            )
                    t = p_xt.tile([128, TQ], F32R, name=f"xt{b}_{ht}", tag="xt")
                    for c in range(NQ2):
                        nc.vector.tensor_copy(t[:, s512(c)], ps[c][:])
                    xt.append(t)

                for ct in range(NCT):
                    ps = [ps_t(f"pso{b}_{ct}_{c}") for c in range(NQ2)]
                    for ht in range(NCT):
                        for c in range(NQ2):
                            nc.tensor.matmul(
                                ps[c][:], wo[ht][:, s128(ct)], xt[ht][:, s512(c)],
                                start=(ht == 0), stop=(ht == NCT - 1),
                            )
                    t = p_out.tile([128, TQ], F32, name=f"ot{b}_{ct}", tag="outt")
                    for c in range(NQ2):
                        nc.scalar.activation(
                            t[:, s512(c)], ps[c][:], AF.Identity,
                            bias=bo_sb[:, ct:ct + 1],
                        )
                    nc.sync.dma_start(out_d.ap()[b, s128(ct), :], t[:])

    nc.compile()
    return nc


def _host_prep(inputs):
    query = np.asarray(inputs["query"], dtype=np.float32)
    keys = np.asarray(inputs["keys"], dtype=np.float32)
    values = np.asarray(inputs["values"], dtype=np.float32)
    tpos = np.asarray(inputs["text_positions"])
    fpos = np.asarray(inputs["frame_positions"])
    mask = np.asarray(inputs["mask"])
    Wq = np.asarray(inputs["Wq"], dtype=np.float32)
    Wk = np.asarray(inputs["Wk"], dtype=np.float32)
    Wv = np.asarray(inputs["Wv"], dtype=np.float32)
    Wo = np.asarray(inputs["Wo"], dtype=np.float32)
    bq = np.asarray(inputs["bq"], dtype=np.float32)
    bk = np.asarray(inputs["bk"], dtype=np.float32)
    bv = np.asarray(inputs["bv"], dtype=np.float32)
    bo = np.asarray(inputs["bo"], dtype=np.float32)

    qT = np.ascontiguousarray(query.transpose(0, 2, 1))
    kT = np.ascontiguousarray(keys.transpose(0, 2, 1))
    vT = np.ascontiguousarray(values.transpose(0, 2, 1))

    # positional-encoding tables (shared across batch when positions agree)
    fshared = bool(np.all(fpos == fpos[0:1]))
    tshared = bool(np.all(tpos == tpos[0:1]))
    fp = fpos[0:1] if fshared else fpos
    tp = tpos[0:1] if tshared else tpos
    peq = np.stack([np.ascontiguousarray(_sin_pos_enc(p, QUERY_POS_RATE, CH).T)
                    for p in fp])
    pek = np.stack([np.ascontiguousarray(_sin_pos_enc(p, KEY_POS_RATE, CH).T)
                    for p in tp])

    mb = np.where(mask, np.float32(MASK_NEG), np.float32(0.0)).astype(np.float32)
    wo_s = (Wo * np.float32(OUT_SCALE)).astype(np.float32)
    bo_s = (np.float32(OUT_SCALE) * (bv @ Wo) + bo).astype(np.float32)
    ones = np.ones((128, 128), dtype=np.float32)

    shared = {
        "wq": Wq, "wk": Wk, "wv": Wv, "wo": wo_s,
        "bq": bq, "bk": bk, "bo": bo_s, "ones": ones,
    }
    in_maps = []
    for c in range(N_CORES):
        sl = slice(c * BPC, (c + 1) * BPC)
        m = dict(shared)
        m["qT"] = qT[sl]
        m["kT"] = kT[sl]
        m["vT"] = vT[sl]
        m["peq"] = peq if fshared else peq[sl]
        m["pek"] = pek if tshared else pek[sl]
        m["mb"] = mb[sl]
        in_maps.append(m)
    return in_maps, fshared, tshared


def kernel(**inputs):
    global _LAST_EXEC_NS, _LAST_RES
    in_maps, fshared, tshared = _host_prep(inputs)
    nc = _build_program(
        BPC,
        1 if fshared else BPC,
        1 if tshared else BPC,
    )
    trace = bool(int(os.environ.get("KERNEL_PROFILE", "0")))
    res = run_bass_kernel_spmd(nc, in_maps, list(range(N_CORES)), trace=trace)
    _LAST_EXEC_NS = res.exec_time_ns
    _LAST_RES = res

    attn = np.empty((B, TQ, TK), dtype=np.float32)
    out = np.empty((B, TQ, CH), dtype=np.float32)
    for c in range(N_CORES):
        r = res.results[c]
        sl = slice(c * BPC, (c + 1) * BPC)
        attn[sl] = r["attnT"].transpose(0, 2, 1)
        out[sl] = r["outT"].transpose(0, 2, 1)
    return out, attn
